# revision 1
# baseline (speedup 1.0000x reference)
"""Scatter-add of active-site feature rows into a dense (B, L, C) output,
distributed over 8 NeuronCores (data-parallel over the batch axis).

Strategy: core m owns flat output positions [m*8192, (m+1)*8192). On the
host we bucket each core's active rows by 128-position output block and pad
every block to a uniform capacity K (runtime max occupancy). On device each
block's output tile is computed as a one-hot matmul:

    out[p, c] = sum_k 1{lidx[k] == p} * feats[k, c]

which sums duplicate indices exactly (fp32 PSUM accumulation) and writes
exact zeros for untouched positions, so every output element is produced by
the kernel (no reliance on pre-zeroed output buffers).
"""

import numpy as np

import concourse.bacc as bacc
import concourse.mybir as mybir
import concourse.tile as tile
from concourse.bass_utils import run_bass_kernel_spmd

N_CORES = 8
B = 16
L = 4096
C = 512
POS_PER_CORE = B * L // N_CORES  # 8192
NBLK = POS_PER_CORE // 128  # 64 blocks of 128 positions per core
G = 8  # blocks per DMA group (16 KB/partition per transfer)

_PROGRAM_CACHE: dict = {}


def _build_program(CH: int, Kc: int):
    f32 = mybir.dt.float32
    nc = bacc.Bacc(
        "TRN2",
        target_bir_lowering=False,
        debug=False,
        enable_asserts=False,
        num_devices=N_CORES,
    )
    feats_d = nc.dram_tensor("feats", [CH, Kc, NBLK * C], f32, kind="ExternalInput")
    lidx_d = nc.dram_tensor("lidx", [CH, Kc, NBLK], f32, kind="ExternalInput")
    iota_d = nc.dram_tensor("iota", [128, 128], f32, kind="ExternalInput")
    out_d = nc.dram_tensor("out", [POS_PER_CORE, C], f32, kind="ExternalOutput")

    eq = mybir.AluOpType.is_equal

    with tile.TileContext(nc) as tc:
        with (
            tc.tile_pool(name="const", bufs=1) as constp,
            tc.tile_pool(name="fpool", bufs=3) as fpool,
            tc.tile_pool(name="opool", bufs=3) as opool,
            tc.tile_pool(name="mpool", bufs=6) as mpool,
            tc.tile_pool(name="psum", bufs=8, space="PSUM") as pspool,
        ):
            iota_t = constp.tile([128, 128], f32)
            nc.sync.dma_start(iota_t[:], iota_d.ap())
            lidx_t = constp.tile([Kc, CH * NBLK], f32)
            for ch in range(CH):
                nc.sync.dma_start(
                    lidx_t[:, ch * NBLK : (ch + 1) * NBLK], lidx_d.ap()[ch]
                )

            out_v = out_d.ap().rearrange("(nb p) c -> nb p c", p=128)  # [64,128,C]
            for g in range(NBLK // G):
                ftiles = []
                for ch in range(CH):
                    ft = fpool.tile([Kc, G * C], f32, tag="ft")
                    nc.sync.dma_start(
                        ft[:], feats_d.ap()[ch, :, g * G * C : (g + 1) * G * C]
                    )
                    ftiles.append(ft)
                ot = opool.tile([128, G * C], f32)
                for j in range(G):
                    b = g * G + j
                    ps = pspool.tile([128, C], f32)
                    for ch in range(CH):
                        m = mpool.tile([Kc, 128], f32)
                        nc.vector.tensor_scalar(
                            m[:],
                            iota_t[:Kc, :],
                            lidx_t[:, ch * NBLK + b : ch * NBLK + b + 1],
                            None,
                            op0=eq,
                        )
                        nc.tensor.matmul(
                            ps[:],
                            m[:],
                            ftiles[ch][:, j * C : (j + 1) * C],
                            start=(ch == 0),
                            stop=(ch == CH - 1),
                        )
                    if j % 2 == 0:
                        nc.scalar.copy(ot[:, j * C : (j + 1) * C], ps[:])
                    else:
                        nc.vector.tensor_copy(ot[:, j * C : (j + 1) * C], ps[:])
                dst = out_v[g * G : (g + 1) * G].rearrange("nb p c -> p nb c")
                src = ot[:].rearrange("p (nb c) -> p nb c", c=C)
                nc.sync.dma_start(dst, src)

    nc.compile()
    return nc


def _prepare_inputs(input_features, site_indices):
    feats = np.ascontiguousarray(np.asarray(input_features, dtype=np.float32))
    idx = np.asarray(site_indices).astype(np.int64)
    n = idx.shape[0]
    assert feats.shape == (n, C)

    gblk = (idx >> 7).astype(np.int64)  # global 128-position block id, 0..511
    lpos = (idx & 127).astype(np.int64)  # position within block

    order = np.argsort(gblk, kind="stable")
    counts = np.bincount(gblk, minlength=N_CORES * NBLK)
    K = int(counts.max())

    CH = (K + 127) // 128
    Kc = -(-K // CH)  # ceil
    Kc = (Kc + 3) & ~3  # round up to multiple of 4

    starts = np.zeros(N_CORES * NBLK, dtype=np.int64)
    np.cumsum(counts[:-1], out=starts[1:])
    slot = np.arange(n, dtype=np.int64) - np.repeat(starts, counts)

    g_sorted = gblk[order]
    core_s = g_sorted >> 6
    blk_s = g_sorted & 63
    ch_s = slot // Kc
    k_s = slot - ch_s * Kc

    feats_pack = np.zeros((N_CORES, CH, Kc, NBLK, C), dtype=np.float32)
    lidx_pack = np.full((N_CORES, CH, Kc, NBLK), -1.0, dtype=np.float32)
    feats_pack[core_s, ch_s, k_s, blk_s, :] = feats[order]
    lidx_pack[core_s, ch_s, k_s, blk_s] = lpos[order].astype(np.float32)

    iota = np.tile(np.arange(128, dtype=np.float32), (128, 1))

    in_maps = [
        {
            "feats": feats_pack[c].reshape(CH, Kc, NBLK * C),
            "lidx": lidx_pack[c],
            "iota": iota,
        }
        for c in range(N_CORES)
    ]
    return in_maps, CH, Kc


def run(input_features, site_indices, trace: bool = False):
    in_maps, CH, Kc = _prepare_inputs(input_features, site_indices)
    key = (CH, Kc)
    if key not in _PROGRAM_CACHE:
        _PROGRAM_CACHE[key] = _build_program(CH, Kc)
    nc = _PROGRAM_CACHE[key]
    res = run_bass_kernel_spmd(nc, in_maps, list(range(N_CORES)), trace=trace)
    out = np.concatenate([res.results[c]["out"] for c in range(N_CORES)], axis=0)
    return out.reshape(B, L, C), res


def kernel(input_features, site_indices, batch_size, length):
    assert int(batch_size) == B and int(length) == L
    out, _ = run(input_features, site_indices, trace=False)
    return out
